# revision 11
# baseline (speedup 1.0000x reference)
"""Non-local (cosine-similarity attention) block on 8 Trainium2 NeuronCores.

Data-parallel over batch: core i handles batch element i.
Per-core program (tokens T=4096, channels C=256):
  qg = query @ wg^T + bg ; kt = ep @ wt^T + bt ; vp = value @ wa^T + ba
  qn = l2norm(qg); kn = l2norm(kt)
  P  = exp(qn @ kn^T)          (cosine sims in [-1,1] -> no max subtraction)
  Z  = rowsum(P)               (from activation accum_out)
  cor = P / Z                  (output 2)
  out = (P @ vp) / Z
  out1 = query + out @ wo^T + bo   (output 1) ; vp is output 3
"""

import numpy as np

import concourse.bacc as bacc
import concourse.bass as bass
import concourse.mybir as mybir
import concourse.tile as tile
from concourse import bass_utils
from concourse.masks import make_identity

F32 = mybir.dt.float32
F16 = mybir.dt.float16
AF = mybir.ActivationFunctionType
TS = bass.ts

C = 256
NCO = 2  # channel chunks of 128


def build_program(T=4096, loops=1):
    """Build the per-core Bass program. T = tokens (hw).

    loops>1 unrolls the whole computation multiple times in one NEFF
    (used only for timing measurements; outputs are identical).
    """
    nc = bacc.Bacc(
        "TRN2", target_bir_lowering=False, debug=False, enable_asserts=False
    )
    NT = T // 128  # token tiles
    SCH = 1024 if T % 1024 == 0 else 512  # S-chunk width (PSUM resident)
    NS = T // SCH  # S chunks per q-tile
    NG = max(1, T // 1024)  # transpose evac groups (1024 cols each)
    GW = T // NG  # group width in k

    query = nc.dram_tensor("query", [T, C], F32, kind="ExternalInput").ap()
    ep = nc.dram_tensor("ep", [T, C], F32, kind="ExternalInput").ap()
    value = nc.dram_tensor("value", [T, C], F32, kind="ExternalInput").ap()
    wdr = {}
    for w in ("wg", "wt", "wa", "wo"):
        wdr[w] = nc.dram_tensor(w, [C, C], F32, kind="ExternalInput").ap()
    for b in ("bg", "bt", "ba", "bo"):
        wdr[b] = nc.dram_tensor(b, [C], F32, kind="ExternalInput").ap()
    out1 = nc.dram_tensor("out1", [T, C], F32, kind="ExternalOutput").ap()
    vpo = nc.dram_tensor("vpo", [T, C], F32, kind="ExternalOutput").ap()
    cor = nc.dram_tensor("cor", [T, T], F32, kind="ExternalOutput").ap()

    with tile.TileContext(nc) as tc:
        for li in range(loops):
            _body(
                nc, tc, T, NT, SCH, NS, NG, GW, query, ep, value, wdr,
                out1, vpo, cor, sfx=f"_l{li}" if loops > 1 else "",
            )
    nc.compile()
    return nc


def _setup_weights(nc, tc, persist, id_bf, wdr, sfx=""):
    """Load weights, cast fp16, transpose to [i, o]; biases + ba broadcast."""
    wT = {}
    bias_sb = {}
    with tc.tile_pool(name="wsetup" + sfx, bufs=2) as wpool, tc.tile_pool(
        name="wpsum" + sfx, bufs=2, space="PSUM"
    ) as wpsum:
        for wname in ("wg", "wt", "wa", "wo"):
            w_f = wpool.tile([128, NCO, C], F32, tag="w_f")
            w_b = wpool.tile([128, NCO, C], F16, tag="w_b")
            for oj in range(NCO):
                nc.sync.dma_start(out=w_f[:, oj, :], in_=wdr[wname][TS(oj, 128), :])
                nc.vector.tensor_copy(w_b[:, oj, :], w_f[:, oj, :])
            wt_sb = persist.tile([128, NCO, C], F16, tag=f"{wname}T")
            for oj in range(NCO):
                for ic in range(NCO):
                    ps = wpsum.tile([128, 128], F16, tag="wtp")
                    nc.tensor.transpose(ps, w_b[:, oj, TS(ic, 128)], id_bf)
                    nc.vector.tensor_copy(wt_sb[:, ic, TS(oj, 128)], ps)
            wT[wname] = wt_sb
        for bname in ("bg", "bt", "bo"):
            b_sb = persist.tile([128, NCO], F32, tag=f"{bname}_sb")
            for oj in range(NCO):
                nc.sync.dma_start(
                    out=b_sb[:, oj : oj + 1],
                    in_=wdr[bname][TS(oj, 128)].rearrange("(p one) -> p one", one=1),
                )
            bias_sb[bname] = b_sb
        # ba varies along the free dim -> broadcast to all 128 partitions via PE
        ba_row = wpool.tile([1, C], F32, tag="ba_row")
        nc.sync.dma_start(
            out=ba_row, in_=wdr["ba"].rearrange("(one c) -> one c", one=1)
        )
        ones_f1 = wpool.tile([1, 128], F32, tag="ones_f1")
        nc.vector.memset(ones_f1, 1.0)
        ba_ps = wpsum.tile([128, C], F32, tag="ba_ps")
        nc.tensor.matmul(ba_ps, ones_f1, ba_row, start=True, stop=True)
        ba_bc = persist.tile([128, C], F32, tag="ba_bc")
        nc.vector.tensor_copy(ba_bc, ba_ps)
    return wT, bias_sb, ba_bc


def _body(nc, tc, T, NT, SCH, NS, NG, GW, query, ep, value, wdr, out1, vpo, cor, sfx=""):
    with tc.tile_pool(name="persist" + sfx, bufs=1) as persist:
        id_bf = persist.tile([128, 128], F16, tag="id_bf")
        make_identity(nc, id_bf)
        id_f32 = persist.tile([128, 128], F32, tag="id_f32")
        make_identity(nc, id_f32)
        ones_bf = persist.tile([128, 128], F16, tag="ones_bf")
        nc.vector.memset(ones_bf, 1.0)

        wT, bias_sb, ba_bc = _setup_weights(nc, tc, persist, id_bf, wdr, sfx)
        out_bf = persist.tile([128, NT, C], F16, tag="out_bf")

        with tc.tile_pool(name="mid" + sfx, bufs=1) as mid:
            qnT = mid.tile([128, NCO, T], F16, tag="qnT")
            knT = mid.tile([128, NCO, T], F16, tag="knT")
            vpb = mid.tile([128, NT, C], F16, tag="vpb")

            with tc.tile_pool(name="acts" + sfx, bufs=1) as acts:
                qT = acts.tile([128, NCO, T], F16, tag="qT")
                eT = acts.tile([128, NCO, T], F16, tag="eT")
                vT = acts.tile([128, NCO, T], F16, tag="vT")

                # cast-load natural layout, PE-transpose to [c, T]
                with tc.tile_pool(name="inat" + sfx, bufs=2) as inat, tc.tile_pool(
                    name="ipsum" + sfx, bufs=3, space="PSUM"
                ) as ipsum:
                    for src, dst in ((query, qT), (ep, eT), (value, vT)):
                        nat = inat.tile([128, NT, C], F16, tag="nat")
                        nc.gpsimd.dma_start(
                            out=nat, in_=src.rearrange("(t p) c -> p t c", p=128)
                        )
                        for ic in range(NCO):
                            for tg in range(NT // 8):
                                ps = ipsum.tile([128, 1024], F16, tag="itp")
                                for j in range(8):
                                    ti = tg * 8 + j
                                    nc.tensor.transpose(
                                        ps[:, TS(j, 128)],
                                        nat[:, ti, TS(ic, 128)],
                                        id_bf,
                                    )
                                nc.vector.tensor_copy(dst[:, ic, TS(tg, 1024)], ps)

                # vp projection (natural layout): exact f32 out + fp16 copy
                with tc.tile_pool(name="vpool" + sfx, bufs=1) as vpool, tc.tile_pool(
                    name="vpsum" + sfx, bufs=3, space="PSUM"
                ) as vpsum:
                    vp_all = vpool.tile([128, NT, C], F32, tag="vp_all")
                    for ti in range(NT):
                        ps = vpsum.tile([128, C], F32, tag="vp_ps")
                        for ic in range(NCO):
                            nc.tensor.matmul(
                                ps,
                                vT[:, ic, TS(ti, 128)],
                                wT["wa"][:, ic, :],
                                start=(ic == 0),
                                stop=(ic == NCO - 1),
                            )
                        nc.vector.tensor_add(vp_all[:, ti, :], ps, ba_bc)
                        nc.vector.tensor_copy(vpb[:, ti, :], vp_all[:, ti, :])
                    nc.sync.dma_start(
                        out=vpo.rearrange("(t p) c -> p t c", p=128), in_=vp_all
                    )

                # qg / kt projections + l2 normalize -> qnT / knT (fp16)
                NQC = T // 512
                with tc.tile_pool(name="npool" + sfx, bufs=1) as npool, tc.tile_pool(
                    name="npsum" + sfx, bufs=4, space="PSUM"
                ) as npsum:
                    for wname, bname, src, dst in (
                        ("wg", "bg", qT, qnT),
                        ("wt", "bt", eT, knT),
                    ):
                        gT = npool.tile([128, NCO, T], F16, tag="gT")
                        for oj in range(NCO):
                            for qc in range(NQC):
                                ps = npsum.tile([128, 512], F32, tag="proj_ps")
                                for ic in range(NCO):
                                    nc.tensor.matmul(
                                        ps,
                                        wT[wname][:, ic, TS(oj, 128)],
                                        src[:, ic, TS(qc, 512)],
                                        start=(ic == 0),
                                        stop=(ic == NCO - 1),
                                    )
                                nc.scalar.activation(
                                    gT[:, oj, TS(qc, 512)],
                                    ps,
                                    AF.Identity,
                                    bias=bias_sb[bname][:, oj : oj + 1],
                                )
                        sq = npool.tile([128, NCO, T], F16, tag="sq")
                        for oj in range(NCO):
                            nc.vector.tensor_mul(
                                sq[:, oj, :], gT[:, oj, :], gT[:, oj, :]
                            )
                        rs = npool.tile([128, T], F32, tag="rs")
                        for qc in range(NQC):
                            nps = npsum.tile([128, 512], F32, tag="norm_ps")
                            for oj in range(NCO):
                                nc.tensor.matmul(
                                    nps,
                                    ones_bf,
                                    sq[:, oj, TS(qc, 512)],
                                    start=(oj == 0),
                                    stop=(oj == NCO - 1),
                                )
                            rcp = npool.tile([128, 512], F32, tag="rcp")
                            nc.vector.reciprocal(rcp, nps)
                            nc.scalar.sqrt(rs[:, TS(qc, 512)], rcp)
                        for oj in range(NCO):
                            nc.vector.tensor_mul(dst[:, oj, :], gT[:, oj, :], rs)

            # ---- main attention loop over q tiles ----
            with tc.tile_pool(name="mloop" + sfx, bufs=2) as mpool, tc.tile_pool(
                name="zpool" + sfx, bufs=3
            ) as zpool, tc.tile_pool(
                name="spsum" + sfx, bufs=2, space="PSUM"
            ) as spsum, tc.tile_pool(
                name="tpsum" + sfx, bufs=2, space="PSUM"
            ) as tpsum, tc.tile_pool(
                name="pvpsum" + sfx, bufs=2, space="PSUM"
            ) as pvpsum:
                for qi in range(NT):
                    pq = mpool.tile([128, T], F16, tag="pq")
                    zacc = zpool.tile([128, NS], F32, tag="zacc")
                    for sc in range(NS):
                        sps = spsum.tile([128, SCH], F32, tag="sps")
                        # ic outer: lhsT stays loaded across the n2 chunks
                        for ic in range(NCO):
                            for n2 in range(SCH // 512):
                                nc.tensor.matmul(
                                    sps[:, TS(n2, 512)],
                                    qnT[:, ic, TS(qi, 128)],
                                    knT[
                                        :,
                                        ic,
                                        sc * SCH + n2 * 512 : sc * SCH + (n2 + 1) * 512,
                                    ],
                                    start=(ic == 0),
                                    stop=(ic == NCO - 1),
                                )
                        nc.scalar.activation(
                            pq[:, TS(sc, SCH)],
                            sps,
                            AF.Exp,
                            accum_out=zacc[:, sc : sc + 1],
                        )
                    rz = zpool.tile([128, 1], F32, tag="rz")
                    zs = zpool.tile([128, 1], F32, tag="zs")
                    nc.vector.tensor_reduce(
                        zs, zacc, axis=mybir.AxisListType.X, op=mybir.AluOpType.add
                    )
                    nc.vector.reciprocal(rz, zs)
                    # cor_map row block: P * (1/Z) fp16, cast-DMA to f32 DRAM
                    corm = mpool.tile([128, T], F16, tag="corm")
                    nc.vector.tensor_scalar_mul(corm, pq, rz)
                    nc.gpsimd.dma_start(out=cor[TS(qi, 128), :], in_=corm)
                    # transpose P chunks; PV accumulate
                    pvps = pvpsum.tile([128, C], F32, tag="pvps")
                    for g in range(NG):
                        tps = tpsum.tile([128, GW], F16, tag="tps")
                        for j in range(GW // 128):
                            nc.tensor.transpose(
                                tps[:, TS(j, 128)],
                                pq[:, g * GW + j * 128 : g * GW + (j + 1) * 128],
                                id_bf,
                            )
                        pt = mpool.tile([128, GW], F16, tag="pt")
                        if g % 2 == 0:
                            nc.vector.tensor_copy(pt, tps)
                        else:
                            nc.scalar.copy(pt, tps)
                        for j in range(GW // 128):
                            kc = g * (GW // 128) + j
                            nc.tensor.matmul(
                                pvps,
                                pt[:, TS(j, 128)],
                                vpb[:, kc, :],
                                start=(kc == 0),
                                stop=(kc == NT - 1),
                            )
                    nc.vector.tensor_scalar_mul(out_bf[:, qi, :], pvps, rz)

        # ---- final: out1 = query + out @ wo^T + bo ----
        with tc.tile_pool(name="fbig" + sfx, bufs=1) as fbig, tc.tile_pool(
            name="fpool" + sfx, bufs=3
        ) as fpool, tc.tile_pool(name="fpsum" + sfx, bufs=2, space="PSUM") as fpsum:
            oT = fbig.tile([128, NCO, T], F16, tag="oT")
            for ic in range(NCO):
                for tg in range(NT // 8):
                    ps = fpsum.tile([128, 1024], F16, tag="ftp")
                    for j in range(8):
                        ti = tg * 8 + j
                        nc.tensor.transpose(
                            ps[:, TS(j, 128)], out_bf[:, ti, TS(ic, 128)], id_bf
                        )
                    nc.vector.tensor_copy(oT[:, ic, TS(tg, 1024)], ps)
            o1T = fbig.tile([128, NCO, T], F32, tag="o1T")
            for oj in range(NCO):
                for qc in range(T // 512):
                    ps = fpsum.tile([128, 512], F32, tag="fproj")
                    for ic in range(NCO):
                        nc.tensor.matmul(
                            ps,
                            wT["wo"][:, ic, TS(oj, 128)],
                            oT[:, ic, TS(qc, 512)],
                            start=(ic == 0),
                            stop=(ic == NCO - 1),
                        )
                    nc.scalar.activation(
                        o1T[:, oj, TS(qc, 512)],
                        ps,
                        AF.Identity,
                        bias=bias_sb["bo"][:, oj : oj + 1],
                    )
            qall = fbig.tile([128, NT, C], F32, tag="qall")
            nc.sync.dma_start(
                out=qall, in_=query.rearrange("(t p) c -> p t c", p=128)
            )
            o1_all = fbig.tile([128, NT, C], F32, tag="o1_all")
            for ti in range(NT):
                ps = fpsum.tile([128, C], F32, tag="fback")
                for oj in range(NCO):
                    nc.tensor.transpose(
                        ps[:, TS(oj, 128)], o1T[:, oj, TS(ti, 128)], id_f32
                    )
                nc.vector.tensor_add(o1_all[:, ti, :], ps, qall[:, ti, :])
            nc.sync.dma_start(
                out=out1.rearrange("(t p) c -> p t c", p=128), in_=o1_all
            )


_PROGRAM = None


def _get_program():
    global _PROGRAM
    if _PROGRAM is None:
        _PROGRAM = build_program(4096)
    return _PROGRAM


def kernel(**inputs):
    B, H, W, Cc = 8, 64, 64, 256
    T = H * W
    nc = _get_program()
    q = np.ascontiguousarray(
        np.asarray(inputs["query"], dtype=np.float32).reshape(B, T, Cc)
    )
    e = np.ascontiguousarray(
        np.asarray(inputs["ep"], dtype=np.float32).reshape(B, T, Cc)
    )
    v = np.ascontiguousarray(
        np.asarray(inputs["value"], dtype=np.float32).reshape(B, T, Cc)
    )
    wb = {
        k: np.ascontiguousarray(np.asarray(inputs[k], dtype=np.float32))
        for k in ("wg", "bg", "wt", "bt", "wa", "ba", "wo", "bo")
    }
    in_maps = [{"query": q[i], "ep": e[i], "value": v[i], **wb} for i in range(B)]
    res = bass_utils.run_bass_kernel_spmd(nc, in_maps, core_ids=list(range(B)))
    o1 = np.stack([res.results[i]["out1"] for i in range(B)]).reshape(B, H, W, Cc)
    vp = np.stack([res.results[i]["vpo"] for i in range(B)])
    cm = np.stack([res.results[i]["cor"] for i in range(B)])
    return o1.astype(np.float32), vp.astype(np.float32), cm.astype(np.float32)


# revision 12
# speedup vs baseline: 6.4430x; 6.4430x over previous
"""Non-local (cosine-similarity attention) block on 8 Trainium2 NeuronCores.

Data-parallel over batch: core i handles batch element i.
Per-core program (tokens T=4096, channels C=256):
  qg = query @ wg^T + bg ; kt = ep @ wt^T + bt ; vp = value @ wa^T + ba
  qn = l2norm(qg); kn = l2norm(kt)
  P  = exp(qn @ kn^T)          (cosine sims in [-1,1] -> no max subtraction)
  Z  = rowsum(P)               (from activation accum_out)
  cor = P / Z                  (output 2)
  out = (P @ vp) / Z
  out1 = query + out @ wo^T + bo   (output 1) ; vp is output 3
"""

import numpy as np

import concourse.bacc as bacc
import concourse.bass as bass
import concourse.mybir as mybir
import concourse.tile as tile
from concourse import bass_utils
from concourse.masks import make_identity

F32 = mybir.dt.float32
F16 = mybir.dt.float16
AF = mybir.ActivationFunctionType
TS = bass.ts

C = 256
NCO = 2  # channel chunks of 128


def build_program(T=4096, loops=1):
    """Build the per-core Bass program. T = tokens (hw).

    loops>1 unrolls the whole computation multiple times in one NEFF
    (used only for timing measurements; outputs are identical).
    """
    nc = bacc.Bacc(
        "TRN2", target_bir_lowering=False, debug=False, enable_asserts=False
    )
    NT = T // 128  # token tiles
    SCH = 1024 if T % 1024 == 0 else 512  # S-chunk width (PSUM resident)
    NS = T // SCH  # S chunks per q-tile
    NG = max(1, T // 1024)  # transpose evac groups (1024 cols each)
    GW = T // NG  # group width in k

    query = nc.dram_tensor("query", [T, C], F32, kind="ExternalInput").ap()
    ep = nc.dram_tensor("ep", [T, C], F32, kind="ExternalInput").ap()
    value = nc.dram_tensor("value", [T, C], F32, kind="ExternalInput").ap()
    wdr = {}
    for w in ("wg", "wt", "wa", "wo"):
        wdr[w] = nc.dram_tensor(w, [C, C], F32, kind="ExternalInput").ap()
    for b in ("bg", "bt", "ba", "bo"):
        wdr[b] = nc.dram_tensor(b, [C], F32, kind="ExternalInput").ap()
    out1 = nc.dram_tensor("out1", [T, C], F32, kind="ExternalOutput").ap()
    vpo = nc.dram_tensor("vpo", [T, C], F32, kind="ExternalOutput").ap()
    cor = nc.dram_tensor("cor", [T, T], F32, kind="ExternalOutput").ap()

    with tile.TileContext(nc) as tc:
        for li in range(loops):
            _body(
                nc, tc, T, NT, SCH, NS, NG, GW, query, ep, value, wdr,
                out1, vpo, cor, sfx=f"_l{li}" if loops > 1 else "",
            )
    nc.compile()
    return nc


def _setup_weights(nc, tc, persist, id_bf, wdr, sfx=""):
    """Load weights, cast fp16, transpose to [i, o]; biases + ba broadcast."""
    wT = {}
    bias_sb = {}
    with tc.tile_pool(name="wsetup" + sfx, bufs=2) as wpool, tc.tile_pool(
        name="wpsum" + sfx, bufs=2, space="PSUM"
    ) as wpsum:
        for wname in ("wg", "wt", "wa", "wo"):
            w_f = wpool.tile([128, NCO, C], F32, tag="w_f")
            w_b = wpool.tile([128, NCO, C], F16, tag="w_b")
            for oj in range(NCO):
                nc.sync.dma_start(out=w_f[:, oj, :], in_=wdr[wname][TS(oj, 128), :])
                nc.vector.tensor_copy(w_b[:, oj, :], w_f[:, oj, :])
            wt_sb = persist.tile([128, NCO, C], F16, tag=f"{wname}T")
            for oj in range(NCO):
                for ic in range(NCO):
                    ps = wpsum.tile([128, 128], F16, tag="wtp")
                    nc.tensor.transpose(ps, w_b[:, oj, TS(ic, 128)], id_bf)
                    nc.vector.tensor_copy(wt_sb[:, ic, TS(oj, 128)], ps)
            wT[wname] = wt_sb
        for bname in ("bg", "bt", "bo"):
            b_sb = persist.tile([128, NCO], F32, tag=f"{bname}_sb")
            for oj in range(NCO):
                nc.sync.dma_start(
                    out=b_sb[:, oj : oj + 1],
                    in_=wdr[bname][TS(oj, 128)].rearrange("(p one) -> p one", one=1),
                )
            bias_sb[bname] = b_sb
        # ba varies along the free dim -> broadcast to all 128 partitions via PE
        ba_row = wpool.tile([1, C], F32, tag="ba_row")
        nc.sync.dma_start(
            out=ba_row, in_=wdr["ba"].rearrange("(one c) -> one c", one=1)
        )
        ones_f1 = wpool.tile([1, 128], F32, tag="ones_f1")
        nc.vector.memset(ones_f1, 1.0)
        ba_ps = wpsum.tile([128, C], F32, tag="ba_ps")
        nc.tensor.matmul(ba_ps, ones_f1, ba_row, start=True, stop=True)
        ba_bc = persist.tile([128, C], F32, tag="ba_bc")
        nc.vector.tensor_copy(ba_bc, ba_ps)
    return wT, bias_sb, ba_bc


def _body(nc, tc, T, NT, SCH, NS, NG, GW, query, ep, value, wdr, out1, vpo, cor, sfx=""):
    with tc.tile_pool(name="persist" + sfx, bufs=1) as persist:
        id_bf = persist.tile([128, 128], F16, tag="id_bf")
        make_identity(nc, id_bf)
        id_f32 = persist.tile([128, 128], F32, tag="id_f32")
        make_identity(nc, id_f32)
        ones_bf = persist.tile([128, 128], F16, tag="ones_bf")
        nc.vector.memset(ones_bf, 1.0)

        wT, bias_sb, ba_bc = _setup_weights(nc, tc, persist, id_bf, wdr, sfx)
        out_bf = persist.tile([128, NT, C], F16, tag="out_bf")

        with tc.tile_pool(name="mid" + sfx, bufs=1) as mid:
            qnT = mid.tile([128, NCO, T], F16, tag="qnT")
            knT = mid.tile([128, NCO, T], F16, tag="knT")
            vpb = mid.tile([128, NT, C], F16, tag="vpb")

            with tc.tile_pool(name="acts" + sfx, bufs=1) as acts:
                qT = acts.tile([128, NCO, T], F16, tag="qT")
                eT = acts.tile([128, NCO, T], F16, tag="eT")
                vT = acts.tile([128, NCO, T], F16, tag="vT")

                # cast-load natural layout, PE-transpose to [c, T]
                with tc.tile_pool(name="inat" + sfx, bufs=2) as inat, tc.tile_pool(
                    name="ipsum" + sfx, bufs=3, space="PSUM"
                ) as ipsum:
                    for src, dst in ((query, qT), (ep, eT), (value, vT)):
                        nat = inat.tile([128, NT, C], F16, tag="nat")
                        nc.gpsimd.dma_start(
                            out=nat, in_=src.rearrange("(t p) c -> p t c", p=128)
                        )
                        for ic in range(NCO):
                            for tg in range(NT // 8):
                                ps = ipsum.tile([128, 1024], F16, tag="itp")
                                for j in range(8):
                                    ti = tg * 8 + j
                                    nc.tensor.transpose(
                                        ps[:, TS(j, 128)],
                                        nat[:, ti, TS(ic, 128)],
                                        id_bf,
                                    )
                                nc.vector.tensor_copy(dst[:, ic, TS(tg, 1024)], ps)

                # vp projection (natural layout): exact f32 out + fp16 copy
                with tc.tile_pool(name="vpool" + sfx, bufs=1) as vpool, tc.tile_pool(
                    name="vpsum" + sfx, bufs=3, space="PSUM"
                ) as vpsum:
                    vp_all = vpool.tile([128, NT, C], F32, tag="vp_all")
                    for ti in range(NT):
                        ps = vpsum.tile([128, C], F32, tag="vp_ps")
                        for ic in range(NCO):
                            nc.tensor.matmul(
                                ps,
                                vT[:, ic, TS(ti, 128)],
                                wT["wa"][:, ic, :],
                                start=(ic == 0),
                                stop=(ic == NCO - 1),
                            )
                        nc.vector.tensor_add(vp_all[:, ti, :], ps, ba_bc)
                        nc.vector.tensor_copy(vpb[:, ti, :], vp_all[:, ti, :])
                    nc.sync.dma_start(
                        out=vpo.rearrange("(t p) c -> p t c", p=128), in_=vp_all
                    )

                # qg / kt projections + l2 normalize -> qnT / knT (fp16)
                NQC = T // 512
                with tc.tile_pool(name="npool" + sfx, bufs=1) as npool, tc.tile_pool(
                    name="npsum" + sfx, bufs=4, space="PSUM"
                ) as npsum:
                    for wname, bname, src, dst in (
                        ("wg", "bg", qT, qnT),
                        ("wt", "bt", eT, knT),
                    ):
                        gT = npool.tile([128, NCO, T], F16, tag="gT")
                        for oj in range(NCO):
                            for qc in range(NQC):
                                ps = npsum.tile([128, 512], F32, tag="proj_ps")
                                for ic in range(NCO):
                                    nc.tensor.matmul(
                                        ps,
                                        wT[wname][:, ic, TS(oj, 128)],
                                        src[:, ic, TS(qc, 512)],
                                        start=(ic == 0),
                                        stop=(ic == NCO - 1),
                                    )
                                nc.scalar.activation(
                                    gT[:, oj, TS(qc, 512)],
                                    ps,
                                    AF.Identity,
                                    bias=bias_sb[bname][:, oj : oj + 1],
                                )
                        sq = npool.tile([128, NCO, T], F16, tag="sq")
                        for oj in range(NCO):
                            nc.vector.tensor_mul(
                                sq[:, oj, :], gT[:, oj, :], gT[:, oj, :]
                            )
                        rs = npool.tile([128, T], F32, tag="rs")
                        for qc in range(NQC):
                            nps = npsum.tile([128, 512], F32, tag="norm_ps")
                            for oj in range(NCO):
                                nc.tensor.matmul(
                                    nps,
                                    ones_bf,
                                    sq[:, oj, TS(qc, 512)],
                                    start=(oj == 0),
                                    stop=(oj == NCO - 1),
                                )
                            rcp = npool.tile([128, 512], F32, tag="rcp")
                            nc.vector.reciprocal(rcp, nps)
                            nc.scalar.sqrt(rs[:, TS(qc, 512)], rcp)
                        for oj in range(NCO):
                            nc.vector.tensor_mul(dst[:, oj, :], gT[:, oj, :], rs)

            # ---- main attention loop over q tiles ----
            with tc.tile_pool(name="mloop" + sfx, bufs=2) as mpool, tc.tile_pool(
                name="zpool" + sfx, bufs=3
            ) as zpool, tc.tile_pool(
                name="spsum" + sfx, bufs=2, space="PSUM"
            ) as spsum, tc.tile_pool(
                name="tpsum" + sfx, bufs=2, space="PSUM"
            ) as tpsum, tc.tile_pool(
                name="pvpsum" + sfx, bufs=2, space="PSUM"
            ) as pvpsum:
                for qi in range(NT):
                    pq = mpool.tile([128, T], F16, tag="pq")
                    zacc = zpool.tile([128, NS], F32, tag="zacc")
                    for sc in range(NS):
                        sps = spsum.tile([128, SCH], F32, tag="sps")
                        # ic outer: lhsT stays loaded across the n2 chunks
                        for ic in range(NCO):
                            for n2 in range(SCH // 512):
                                nc.tensor.matmul(
                                    sps[:, TS(n2, 512)],
                                    qnT[:, ic, TS(qi, 128)],
                                    knT[
                                        :,
                                        ic,
                                        sc * SCH + n2 * 512 : sc * SCH + (n2 + 1) * 512,
                                    ],
                                    start=(ic == 0),
                                    stop=(ic == NCO - 1),
                                )
                        nc.scalar.activation(
                            pq[:, TS(sc, SCH)],
                            sps,
                            AF.Exp,
                            accum_out=zacc[:, sc : sc + 1],
                        )
                    rz = zpool.tile([128, 1], F32, tag="rz")
                    zs = zpool.tile([128, 1], F32, tag="zs")
                    nc.vector.tensor_reduce(
                        zs, zacc, axis=mybir.AxisListType.X, op=mybir.AluOpType.add
                    )
                    nc.vector.reciprocal(rz, zs)
                    # cor_map row block: P * (1/Z) -> f32, plain HWDGE DMA
                    corm = mpool.tile([128, T], F32, tag="corm")
                    nc.vector.tensor_scalar_mul(corm, pq, rz)
                    nc.sync.dma_start(out=cor[TS(qi, 128), :], in_=corm)
                    # transpose P chunks; PV accumulate
                    pvps = pvpsum.tile([128, C], F32, tag="pvps")
                    for g in range(NG):
                        tps = tpsum.tile([128, GW], F16, tag="tps")
                        for j in range(GW // 128):
                            nc.tensor.transpose(
                                tps[:, TS(j, 128)],
                                pq[:, g * GW + j * 128 : g * GW + (j + 1) * 128],
                                id_bf,
                            )
                        pt = mpool.tile([128, GW], F16, tag="pt")
                        if g % 2 == 0:
                            nc.vector.tensor_copy(pt, tps)
                        else:
                            nc.scalar.copy(pt, tps)
                        for j in range(GW // 128):
                            kc = g * (GW // 128) + j
                            nc.tensor.matmul(
                                pvps,
                                pt[:, TS(j, 128)],
                                vpb[:, kc, :],
                                start=(kc == 0),
                                stop=(kc == NT - 1),
                            )
                    nc.vector.tensor_scalar_mul(out_bf[:, qi, :], pvps, rz)

        # ---- final: out1 = query + out @ wo^T + bo ----
        with tc.tile_pool(name="fbig" + sfx, bufs=1) as fbig, tc.tile_pool(
            name="fpool" + sfx, bufs=3
        ) as fpool, tc.tile_pool(name="fpsum" + sfx, bufs=2, space="PSUM") as fpsum:
            oT = fbig.tile([128, NCO, T], F16, tag="oT")
            for ic in range(NCO):
                for tg in range(NT // 8):
                    ps = fpsum.tile([128, 1024], F16, tag="ftp")
                    for j in range(8):
                        ti = tg * 8 + j
                        nc.tensor.transpose(
                            ps[:, TS(j, 128)], out_bf[:, ti, TS(ic, 128)], id_bf
                        )
                    nc.vector.tensor_copy(oT[:, ic, TS(tg, 1024)], ps)
            o1T = fbig.tile([128, NCO, T], F32, tag="o1T")
            for oj in range(NCO):
                for qc in range(T // 512):
                    ps = fpsum.tile([128, 512], F32, tag="fproj")
                    for ic in range(NCO):
                        nc.tensor.matmul(
                            ps,
                            wT["wo"][:, ic, TS(oj, 128)],
                            oT[:, ic, TS(qc, 512)],
                            start=(ic == 0),
                            stop=(ic == NCO - 1),
                        )
                    nc.scalar.activation(
                        o1T[:, oj, TS(qc, 512)],
                        ps,
                        AF.Identity,
                        bias=bias_sb["bo"][:, oj : oj + 1],
                    )
            qall = fbig.tile([128, NT, C], F32, tag="qall")
            nc.sync.dma_start(
                out=qall, in_=query.rearrange("(t p) c -> p t c", p=128)
            )
            o1_all = fbig.tile([128, NT, C], F32, tag="o1_all")
            for ti in range(NT):
                ps = fpsum.tile([128, C], F32, tag="fback")
                for oj in range(NCO):
                    nc.tensor.transpose(
                        ps[:, TS(oj, 128)], o1T[:, oj, TS(ti, 128)], id_f32
                    )
                nc.vector.tensor_add(o1_all[:, ti, :], ps, qall[:, ti, :])
            nc.sync.dma_start(
                out=out1.rearrange("(t p) c -> p t c", p=128), in_=o1_all
            )


_PROGRAM = None


def _get_program():
    global _PROGRAM
    if _PROGRAM is None:
        _PROGRAM = build_program(4096)
    return _PROGRAM


def kernel(**inputs):
    B, H, W, Cc = 8, 64, 64, 256
    T = H * W
    nc = _get_program()
    q = np.ascontiguousarray(
        np.asarray(inputs["query"], dtype=np.float32).reshape(B, T, Cc)
    )
    e = np.ascontiguousarray(
        np.asarray(inputs["ep"], dtype=np.float32).reshape(B, T, Cc)
    )
    v = np.ascontiguousarray(
        np.asarray(inputs["value"], dtype=np.float32).reshape(B, T, Cc)
    )
    wb = {
        k: np.ascontiguousarray(np.asarray(inputs[k], dtype=np.float32))
        for k in ("wg", "bg", "wt", "bt", "wa", "ba", "wo", "bo")
    }
    in_maps = [{"query": q[i], "ep": e[i], "value": v[i], **wb} for i in range(B)]
    res = bass_utils.run_bass_kernel_spmd(nc, in_maps, core_ids=list(range(B)))
    o1 = np.stack([res.results[i]["out1"] for i in range(B)]).reshape(B, H, W, Cc)
    vp = np.stack([res.results[i]["vpo"] for i in range(B)])
    cm = np.stack([res.results[i]["cor"] for i in range(B)])
    return o1.astype(np.float32), vp.astype(np.float32), cm.astype(np.float32)


# revision 13
# speedup vs baseline: 13.5053x; 2.0961x over previous
"""Non-local (cosine-similarity attention) block on 8 Trainium2 NeuronCores.

Data-parallel over batch: core i handles batch element i.
Per-core program (tokens T=4096, channels C=256):
  qg = query @ wg^T + bg ; kt = ep @ wt^T + bt ; vp = value @ wa^T + ba
  qn = l2norm(qg); kn = l2norm(kt)
  P  = exp(qn @ kn^T)          (cosine sims in [-1,1] -> no max subtraction)
  Z  = rowsum(P)               (from activation accum_out)
  cor = P / Z                  (output 2)
  out = (P @ vp) / Z
  out1 = query + out @ wo^T + bo   (output 1) ; vp is output 3
"""

import numpy as np

import concourse.bacc as bacc
import concourse.bass as bass
import concourse.mybir as mybir
import concourse.tile as tile
from concourse import bass_utils
from concourse.masks import make_identity

F32 = mybir.dt.float32
F16 = mybir.dt.float16
AF = mybir.ActivationFunctionType
TS = bass.ts

C = 256
NCO = 2  # channel chunks of 128


def build_program(T=4096, loops=1):
    """Build the per-core Bass program. T = tokens (hw).

    loops>1 unrolls the whole computation multiple times in one NEFF
    (used only for timing measurements; outputs are identical).
    """
    nc = bacc.Bacc(
        "TRN2", target_bir_lowering=False, debug=False, enable_asserts=False
    )
    NT = T // 128  # token tiles
    SCH = 1024 if T % 1024 == 0 else 512  # S-chunk width (PSUM resident)
    NS = T // SCH  # S chunks per q-tile
    NG = max(1, T // 1024)  # transpose evac groups (1024 cols each)
    GW = T // NG  # group width in k

    query = nc.dram_tensor("query", [T, C], F32, kind="ExternalInput").ap()
    ep = nc.dram_tensor("ep", [T, C], F32, kind="ExternalInput").ap()
    value = nc.dram_tensor("value", [T, C], F32, kind="ExternalInput").ap()
    wdr = {}
    for w in ("wg", "wt", "wa", "wo"):
        wdr[w] = nc.dram_tensor(w, [C, C], F32, kind="ExternalInput").ap()
    for b in ("bg", "bt", "ba", "bo"):
        wdr[b] = nc.dram_tensor(b, [C], F32, kind="ExternalInput").ap()
    out1 = nc.dram_tensor("out1", [T, C], F32, kind="ExternalOutput").ap()
    vpo = nc.dram_tensor("vpo", [T, C], F32, kind="ExternalOutput").ap()
    cor = nc.dram_tensor("cor", [T, T], F32, kind="ExternalOutput").ap()

    with tile.TileContext(nc) as tc:
        for li in range(loops):
            _body(
                nc, tc, T, NT, SCH, NS, NG, GW, query, ep, value, wdr,
                out1, vpo, cor, sfx=f"_l{li}" if loops > 1 else "",
            )
    nc.compile()
    return nc


def _setup_weights(nc, tc, persist, id_bf, wdr, sfx=""):
    """Load weights, cast fp16, transpose to [i, o]; biases + ba broadcast."""
    wT = {}
    bias_sb = {}
    with tc.tile_pool(name="wsetup" + sfx, bufs=2) as wpool, tc.tile_pool(
        name="wpsum" + sfx, bufs=2, space="PSUM"
    ) as wpsum:
        for wname in ("wg", "wt", "wa", "wo"):
            w_f = wpool.tile([128, NCO, C], F32, tag="w_f")
            w_b = wpool.tile([128, NCO, C], F16, tag="w_b")
            for oj in range(NCO):
                nc.sync.dma_start(out=w_f[:, oj, :], in_=wdr[wname][TS(oj, 128), :])
                nc.vector.tensor_copy(w_b[:, oj, :], w_f[:, oj, :])
            wt_sb = persist.tile([128, NCO, C], F16, tag=f"{wname}T")
            for oj in range(NCO):
                for ic in range(NCO):
                    ps = wpsum.tile([128, 128], F16, tag="wtp")
                    nc.tensor.transpose(ps, w_b[:, oj, TS(ic, 128)], id_bf)
                    nc.vector.tensor_copy(wt_sb[:, ic, TS(oj, 128)], ps)
            wT[wname] = wt_sb
        for bname in ("bg", "bt", "bo"):
            b_sb = persist.tile([128, NCO], F32, tag=f"{bname}_sb")
            for oj in range(NCO):
                nc.sync.dma_start(
                    out=b_sb[:, oj : oj + 1],
                    in_=wdr[bname][TS(oj, 128)].rearrange("(p one) -> p one", one=1),
                )
            bias_sb[bname] = b_sb
        # ba varies along the free dim -> broadcast to all 128 partitions via PE
        ba_row = wpool.tile([1, C], F32, tag="ba_row")
        nc.sync.dma_start(
            out=ba_row, in_=wdr["ba"].rearrange("(one c) -> one c", one=1)
        )
        ones_f1 = wpool.tile([1, 128], F32, tag="ones_f1")
        nc.vector.memset(ones_f1, 1.0)
        ba_ps = wpsum.tile([128, C], F32, tag="ba_ps")
        nc.tensor.matmul(ba_ps, ones_f1, ba_row, start=True, stop=True)
        ba_bc = persist.tile([128, C], F32, tag="ba_bc")
        nc.vector.tensor_copy(ba_bc, ba_ps)
    return wT, bias_sb, ba_bc


def _body(nc, tc, T, NT, SCH, NS, NG, GW, query, ep, value, wdr, out1, vpo, cor, sfx=""):
    with tc.tile_pool(name="persist" + sfx, bufs=1) as persist:
        id_bf = persist.tile([128, 128], F16, tag="id_bf")
        make_identity(nc, id_bf)
        id_f32 = persist.tile([128, 128], F32, tag="id_f32")
        make_identity(nc, id_f32)
        ones_bf = persist.tile([128, 128], F16, tag="ones_bf")
        nc.vector.memset(ones_bf, 1.0)

        wT, bias_sb, ba_bc = _setup_weights(nc, tc, persist, id_bf, wdr, sfx)
        out_bf = persist.tile([128, NT, C], F16, tag="out_bf")

        with tc.tile_pool(name="mid" + sfx, bufs=1) as mid:
            qnT = mid.tile([128, NCO, T], F16, tag="qnT")
            knT = mid.tile([128, NCO, T], F16, tag="knT")
            vpb = mid.tile([128, NT, C], F16, tag="vpb")

            with tc.tile_pool(name="acts" + sfx, bufs=1) as acts:
                qT = acts.tile([128, NCO, T], F16, tag="qT")
                eT = acts.tile([128, NCO, T], F16, tag="eT")
                vT = acts.tile([128, NCO, T], F16, tag="vT")

                # cast-load natural layout, PE-transpose to [c, T]
                with tc.tile_pool(name="inat" + sfx, bufs=2) as inat, tc.tile_pool(
                    name="ipsum" + sfx, bufs=3, space="PSUM"
                ) as ipsum:
                    for src, dst in ((query, qT), (ep, eT), (value, vT)):
                        nat = inat.tile([128, NT, C], F16, tag="nat")
                        h = NT // 2
                        src_r = src.rearrange("(t p) c -> p t c", p=128)
                        nc.gpsimd.dma_start(out=nat[:, :h, :], in_=src_r[:, :h, :])
                        nc.gpsimd.dma_start(out=nat[:, h:, :], in_=src_r[:, h:, :])
                        for ic in range(NCO):
                            for tg in range(NT // 8):
                                ps = ipsum.tile([128, 1024], F16, tag="itp")
                                for j in range(8):
                                    ti = tg * 8 + j
                                    nc.tensor.transpose(
                                        ps[:, TS(j, 128)],
                                        nat[:, ti, TS(ic, 128)],
                                        id_bf,
                                    )
                                nc.vector.tensor_copy(dst[:, ic, TS(tg, 1024)], ps)

                # vp projection (natural layout): exact f32 out + fp16 copy
                with tc.tile_pool(name="vpool" + sfx, bufs=1) as vpool, tc.tile_pool(
                    name="vpsum" + sfx, bufs=3, space="PSUM"
                ) as vpsum:
                    vp_all = vpool.tile([128, NT, C], F32, tag="vp_all")
                    for ti in range(NT):
                        ps = vpsum.tile([128, C], F32, tag="vp_ps")
                        for ic in range(NCO):
                            nc.tensor.matmul(
                                ps,
                                vT[:, ic, TS(ti, 128)],
                                wT["wa"][:, ic, :],
                                start=(ic == 0),
                                stop=(ic == NCO - 1),
                            )
                        nc.vector.tensor_add(vp_all[:, ti, :], ps, ba_bc)
                        nc.vector.tensor_copy(vpb[:, ti, :], vp_all[:, ti, :])
                    nc.sync.dma_start(
                        out=vpo.rearrange("(t p) c -> p t c", p=128), in_=vp_all
                    )

                # qg / kt projections + l2 normalize -> qnT / knT (fp16)
                NQC = T // 512
                with tc.tile_pool(name="npool" + sfx, bufs=2) as npool, tc.tile_pool(
                    name="npsum" + sfx, bufs=4, space="PSUM"
                ) as npsum:
                    for wname, bname, src, dst in (
                        ("wg", "bg", qT, qnT),
                        ("wt", "bt", eT, knT),
                    ):
                        gT = npool.tile([128, NCO, T], F16, tag="gT")
                        rs = npool.tile([128, T], F16, tag="rs")
                        for qc in range(NQC):
                            # projection chunk (both oj halves)
                            for oj in range(NCO):
                                ps = npsum.tile([128, 512], F32, tag="proj_ps")
                                for ic in range(NCO):
                                    nc.tensor.matmul(
                                        ps,
                                        wT[wname][:, ic, TS(oj, 128)],
                                        src[:, ic, TS(qc, 512)],
                                        start=(ic == 0),
                                        stop=(ic == NCO - 1),
                                    )
                                nc.scalar.activation(
                                    gT[:, oj, TS(qc, 512)],
                                    ps,
                                    AF.Identity,
                                    bias=bias_sb[bname][:, oj : oj + 1],
                                )
                            # squares + column-sum + rsqrt for this chunk
                            sqc = npool.tile([128, NCO, 512], F16, tag="sqc")
                            for oj in range(NCO):
                                nc.vector.tensor_mul(
                                    sqc[:, oj, :],
                                    gT[:, oj, TS(qc, 512)],
                                    gT[:, oj, TS(qc, 512)],
                                )
                            nps = npsum.tile([128, 512], F32, tag="norm_ps")
                            for oj in range(NCO):
                                nc.tensor.matmul(
                                    nps,
                                    ones_bf,
                                    sqc[:, oj, :],
                                    start=(oj == 0),
                                    stop=(oj == NCO - 1),
                                )
                            rcp = npool.tile([128, 512], F32, tag="rcp")
                            nc.vector.reciprocal(rcp, nps)
                            nc.scalar.sqrt(rs[:, TS(qc, 512)], rcp)
                        for oj in range(NCO):
                            nc.vector.tensor_mul(dst[:, oj, :], gT[:, oj, :], rs)

            # ---- main attention loop over q tiles ----
            with tc.tile_pool(name="mloop" + sfx, bufs=3) as mpool, tc.tile_pool(
                name="zpool" + sfx, bufs=3
            ) as zpool, tc.tile_pool(
                name="spsum" + sfx, bufs=2, space="PSUM"
            ) as spsum, tc.tile_pool(
                name="tpsum" + sfx, bufs=2, space="PSUM"
            ) as tpsum, tc.tile_pool(
                name="pvpsum" + sfx, bufs=2, space="PSUM"
            ) as pvpsum:
                for qi in range(NT):
                    pq = mpool.tile([128, T], F16, tag="pq")
                    zacc = zpool.tile([128, NS], F32, tag="zacc")
                    for sc in range(NS):
                        sps = spsum.tile([128, SCH], F32, tag="sps")
                        # ic outer: lhsT stays loaded across the n2 chunks
                        for ic in range(NCO):
                            for n2 in range(SCH // 512):
                                nc.tensor.matmul(
                                    sps[:, TS(n2, 512)],
                                    qnT[:, ic, TS(qi, 128)],
                                    knT[
                                        :,
                                        ic,
                                        sc * SCH + n2 * 512 : sc * SCH + (n2 + 1) * 512,
                                    ],
                                    start=(ic == 0),
                                    stop=(ic == NCO - 1),
                                )
                        nc.scalar.activation(
                            pq[:, TS(sc, SCH)],
                            sps,
                            AF.Exp,
                            accum_out=zacc[:, sc : sc + 1],
                        )
                    rz = zpool.tile([128, 1], F32, tag="rz")
                    zs = zpool.tile([128, 1], F32, tag="zs")
                    nc.vector.tensor_reduce(
                        zs, zacc, axis=mybir.AxisListType.X, op=mybir.AluOpType.add
                    )
                    nc.vector.reciprocal(rz, zs)
                    # cor_map row block: P * (1/Z) -> f32, plain HWDGE DMA
                    corm = mpool.tile([128, T], F32, tag="corm")
                    nc.vector.tensor_scalar_mul(corm, pq, rz)
                    nc.sync.dma_start(out=cor[TS(qi, 128), :], in_=corm)
                    # transpose P chunks; PV accumulate
                    pvps = pvpsum.tile([128, C], F32, tag="pvps")
                    for g in range(NG):
                        tps = tpsum.tile([128, GW], F16, tag="tps")
                        for j in range(GW // 128):
                            nc.tensor.transpose(
                                tps[:, TS(j, 128)],
                                pq[:, g * GW + j * 128 : g * GW + (j + 1) * 128],
                                id_bf,
                            )
                        pt = mpool.tile([128, GW], F16, tag="pt")
                        if g % 2 == 0:
                            nc.vector.tensor_copy(pt, tps)
                        else:
                            nc.scalar.copy(pt, tps)
                        for j in range(GW // 128):
                            kc = g * (GW // 128) + j
                            nc.tensor.matmul(
                                pvps,
                                pt[:, TS(j, 128)],
                                vpb[:, kc, :],
                                start=(kc == 0),
                                stop=(kc == NT - 1),
                            )
                    nc.vector.tensor_scalar_mul(out_bf[:, qi, :], pvps, rz)

        # ---- final: out1 = query + out @ wo^T + bo ----
        with tc.tile_pool(name="fbig" + sfx, bufs=1) as fbig, tc.tile_pool(
            name="fpool" + sfx, bufs=3
        ) as fpool, tc.tile_pool(name="fpsum" + sfx, bufs=2, space="PSUM") as fpsum:
            oT = fbig.tile([128, NCO, T], F16, tag="oT")
            for ic in range(NCO):
                for tg in range(NT // 8):
                    ps = fpsum.tile([128, 1024], F16, tag="ftp")
                    for j in range(8):
                        ti = tg * 8 + j
                        nc.tensor.transpose(
                            ps[:, TS(j, 128)], out_bf[:, ti, TS(ic, 128)], id_bf
                        )
                    nc.vector.tensor_copy(oT[:, ic, TS(tg, 1024)], ps)
            o1T = fbig.tile([128, NCO, T], F32, tag="o1T")
            for oj in range(NCO):
                for qc in range(T // 512):
                    ps = fpsum.tile([128, 512], F32, tag="fproj")
                    for ic in range(NCO):
                        nc.tensor.matmul(
                            ps,
                            wT["wo"][:, ic, TS(oj, 128)],
                            oT[:, ic, TS(qc, 512)],
                            start=(ic == 0),
                            stop=(ic == NCO - 1),
                        )
                    nc.scalar.activation(
                        o1T[:, oj, TS(qc, 512)],
                        ps,
                        AF.Identity,
                        bias=bias_sb["bo"][:, oj : oj + 1],
                    )
            qall = fbig.tile([128, NT, C], F32, tag="qall")
            nc.sync.dma_start(
                out=qall, in_=query.rearrange("(t p) c -> p t c", p=128)
            )
            o1_all = fbig.tile([128, NT, C], F32, tag="o1_all")
            for ti in range(NT):
                ps = fpsum.tile([128, C], F32, tag="fback")
                for oj in range(NCO):
                    nc.tensor.transpose(
                        ps[:, TS(oj, 128)], o1T[:, oj, TS(ti, 128)], id_f32
                    )
                nc.vector.tensor_add(o1_all[:, ti, :], ps, qall[:, ti, :])
            nc.sync.dma_start(
                out=out1.rearrange("(t p) c -> p t c", p=128), in_=o1_all
            )


_PROGRAM = None


def _get_program():
    global _PROGRAM
    if _PROGRAM is None:
        _PROGRAM = build_program(4096)
    return _PROGRAM


def kernel(**inputs):
    B, H, W, Cc = 8, 64, 64, 256
    T = H * W
    nc = _get_program()
    q = np.ascontiguousarray(
        np.asarray(inputs["query"], dtype=np.float32).reshape(B, T, Cc)
    )
    e = np.ascontiguousarray(
        np.asarray(inputs["ep"], dtype=np.float32).reshape(B, T, Cc)
    )
    v = np.ascontiguousarray(
        np.asarray(inputs["value"], dtype=np.float32).reshape(B, T, Cc)
    )
    wb = {
        k: np.ascontiguousarray(np.asarray(inputs[k], dtype=np.float32))
        for k in ("wg", "bg", "wt", "bt", "wa", "ba", "wo", "bo")
    }
    in_maps = [{"query": q[i], "ep": e[i], "value": v[i], **wb} for i in range(B)]
    res = bass_utils.run_bass_kernel_spmd(nc, in_maps, core_ids=list(range(B)))
    o1 = np.stack([res.results[i]["out1"] for i in range(B)]).reshape(B, H, W, Cc)
    vp = np.stack([res.results[i]["vpo"] for i in range(B)])
    cm = np.stack([res.results[i]["cor"] for i in range(B)])
    return o1.astype(np.float32), vp.astype(np.float32), cm.astype(np.float32)
